# revision 4
# baseline (speedup 1.0000x reference)
"""Causal self-attention (B=4, T=2048, C=768, H=12) on 8 TRN2 NeuronCores.

Sharding: DP=4 over batch x TP=2 over heads (6 heads per core).

v2 design, built from HAM-throttle analysis of the v1 trace:
  - The PE defaults to 1.2 GHz and only reaches 2.4 GHz under *sustained*
    busy-ness (PE_HAM 4096-cycle activity windows).  v1's attention phase
    ran 225us straight at half clock because of micro-stalls.  v2 keeps the
    PE stream dense: S / OT matmuls for the two heads of a pair are
    software-pipelined (OT lags one k-tile behind S), and idle slack is
    filled with the next pair's kqv chains / the finished pair's output
    projection.
  - x^T transposes moved from the (serialized, slow) sync-DMA queue to the
    PE itself (matmul-transpose against identity), with gpsimd draining
    PSUM -> SBUF.  Saves ~110us of front-phase serialization.
  - The causal diagonal mask is accumulated into scores by a PE matmul
    (identity x trimask) instead of a DVE tensor_add: keeps the DVE light
    and removes a cross-engine dependency hop.
  - exp() runs on the scalar engine only (the only engine with ACT tables):
    ~13.4M causal-area elements ~= 100-130us.  Everything else is arranged
    to hide under max(PE, ACT).

Per-core pipeline:
    front:  x tiles (gpsimd cast-DMA) -> PE transpose -> X^T
            V = X^T' W_v + b_v (PE, ones-row bias matmul), K/Q pair 0
    pair p: for qb (512 queries): for kt: S^T(h0), S^T(h1) [+ diag mask],
            exp -> P (bf16), OT(h0,kt-1), OT(h1,kt-1) accumulating
            [O~^T; l] over kt; normalize rows by 1/l (DVE+gpsimd).
            Fillers: kqv chains of pair p+1 (p<2), Y projection of
            completed query blocks (p==2).
    Y: y[qb] = sum_p otn[p]' Wp[p]  (PSUM chain over pairs) -> DMA out.
Host sums the two TP partials per batch and adds b_proj.
"""

import sys

sys.path.insert(0, "/opt/trn_rl_repo")

from contextlib import ExitStack

import numpy as np

import concourse.bass as bass
import concourse.tile as tile
from concourse import bacc
from concourse import mybir
from concourse.bass import ts
from concourse.bass_utils import run_bass_kernel_spmd
from concourse.masks import make_identity

F32 = mybir.dt.float32
BF16 = mybir.dt.bfloat16

B, T, C = 4, 2048, 768
H, D = 12, 64
HL = 6          # heads per core
FL = HL * D     # 384 local feature dim
NCT = C // 128  # 6 contraction tiles
NT = T // 128   # 16 token tiles
NQB = T // 512  # 4 query blocks
NPAIR = HL // 2  # 3 head pairs

MASK_NEG = -30000.0


def build_nc():
    nc = bacc.Bacc()
    x_d = nc.declare_dram_parameter("x", [T, C], F32, isOutput=False)
    wk_d = nc.declare_dram_parameter("wk", [C, FL], F32, isOutput=False)
    wq_d = nc.declare_dram_parameter("wq", [C, FL], F32, isOutput=False)
    wv_d = nc.declare_dram_parameter("wv", [C, FL], F32, isOutput=False)
    wp_d = nc.declare_dram_parameter("wp", [FL, C], F32, isOutput=False)
    bk_d = nc.declare_dram_parameter("bk", [FL], F32, isOutput=False)
    bq_d = nc.declare_dram_parameter("bq", [FL], F32, isOutput=False)
    bv_d = nc.declare_dram_parameter("bv", [FL], F32, isOutput=False)
    mask_d = nc.declare_dram_parameter("mask", [128, 128], F32, isOutput=False)
    y_d = nc.declare_dram_parameter("y", [T, C], F32, isOutput=True)

    with tile.TileContext(nc) as tc, ExitStack() as ctx:
        const = ctx.enter_context(tc.tile_pool(name="const", bufs=1))
        wpool = ctx.enter_context(tc.tile_pool(name="wpool", bufs=1))
        big = ctx.enter_context(tc.tile_pool(name="big", bufs=1))
        xin = ctx.enter_context(tc.tile_pool(name="xin", bufs=6))
        ppool = ctx.enter_context(tc.tile_pool(name="ppool", bufs=6))
        small = ctx.enter_context(tc.tile_pool(name="small", bufs=2))
        ypool = ctx.enter_context(tc.tile_pool(name="ypool", bufs=4))
        psum = ctx.enter_context(tc.tile_pool(name="psum", bufs=2, space="PSUM"))

        # ---- constants ----
        ident = const.tile([128, 128], BF16)
        make_identity(nc, ident)
        # mask[k, q] = 0 where k <= q (causal-valid), else MASK_NEG;
        # accumulated onto diagonal score blocks by a PE matmul vs ident.
        trimask = const.tile([128, 128], BF16)
        ones_sb = const.tile([1, 128], BF16)
        nc.vector.memset(ones_sb, 1.0)
        bk_sb = const.tile([128, NPAIR], F32)
        bq_sb = const.tile([128, NPAIR], F32)
        bv_sb = const.tile([1, FL], BF16)

        # ---- weights (packed, one cast-DMA per matrix) ----
        wk_sb = wpool.tile([128, NCT, FL], BF16)
        wq_sb = wpool.tile([128, NCT, FL], BF16)
        wv_sb = wpool.tile([128, NCT, FL], BF16)
        wp_sb = wpool.tile([128, NPAIR, C], BF16)

        # ---- persistent activations ----
        xt_sb = [
            big.tile([128, T], BF16, tag="xt", bufs=NCT, name=f"xt{ct}")
            for ct in range(NCT)
        ]
        kt_sb = [
            big.tile([128, T], BF16, tag="ktq", bufs=2 * NPAIR, name=f"ktp{i}")
            for i in range(NPAIR)
        ]
        qt_sb = [
            big.tile([128, T], BF16, tag="ktq", bufs=2 * NPAIR, name=f"qtp{i}")
            for i in range(NPAIR)
        ]
        v_sb = [
            big.tile([128, HL, D + 1], BF16, tag="v", bufs=NT, name=f"v{t}")
            for t in range(NT)
        ]
        otn_sb = [
            big.tile([128, T], BF16, tag="otn", bufs=NPAIR, name=f"otn{i}")
            for i in range(NPAIR)
        ]

        # ---- input DMAs (gpsimd SWDGE: casts f32 -> bf16 in flight) ----
        # x tiles first (transposes gate everything), weights interleaved.
        xb_sb = [
            xin.tile([128, C], BF16, tag="xb", bufs=NT, name=f"xb{t}")
            for t in range(NT)
        ]
        for t in range(4):
            nc.gpsimd.dma_start(out=xb_sb[t], in_=x_d[ts(t, 128), :])
        nc.gpsimd.dma_start(
            out=wv_sb, in_=wv_d.rearrange("(a p) f -> p a f", p=128)
        )
        nc.gpsimd.dma_start(
            out=wk_sb, in_=wk_d.rearrange("(a p) f -> p a f", p=128)
        )
        nc.gpsimd.dma_start(out=bv_sb, in_=bv_d.rearrange("(o f) -> o f", o=1))
        for t in range(4, 8):
            nc.gpsimd.dma_start(out=xb_sb[t], in_=x_d[ts(t, 128), :])
        nc.gpsimd.dma_start(
            out=wq_sb, in_=wq_d.rearrange("(a p) f -> p a f", p=128)
        )
        nc.gpsimd.dma_start(out=bk_sb, in_=bk_d.rearrange("(i p) -> p i", p=128))
        nc.gpsimd.dma_start(out=bq_sb, in_=bq_d.rearrange("(i p) -> p i", p=128))
        for t in range(8, 12):
            nc.gpsimd.dma_start(out=xb_sb[t], in_=x_d[ts(t, 128), :])
        nc.gpsimd.dma_start(
            out=wp_sb, in_=wp_d.rearrange("(a p) f -> p a f", p=128)
        )
        nc.gpsimd.dma_start(out=trimask, in_=mask_d[:, :])
        for t in range(12, 16):
            nc.gpsimd.dma_start(out=xb_sb[t], in_=x_d[ts(t, 128), :])

        # ---- helpers ----
        def kq_chain(pair, n, which):
            """One K- or Q-pair projection chain for token block n (512)."""
            w_src, dest, bias = (
                (wk_sb, kt_sb, bk_sb) if which == "k" else (wq_sb, qt_sb, bq_sb)
            )
            ps = psum.tile(
                [128, 512], F32, tag="sps", bufs=4, name=f"kq{which}{pair}_{n}"
            )
            for ct in range(NCT):
                nc.tensor.matmul(
                    out=ps,
                    lhsT=w_src[:, ct, ts(pair, 128)],
                    rhs=xt_sb[ct][:, ts(n, 512)],
                    start=(ct == 0),
                    stop=(ct == NCT - 1),
                )
            nc.vector.tensor_scalar_add(
                out=dest[pair][:, ts(n, 512)], in0=ps, scalar1=bias[:, pair : pair + 1]
            )

        def v_chain(t):
            """V (natural layout) for token tile t, bias via ones-row matmul."""
            psv = psum.tile([128, FL], F32, tag="ot", bufs=2, name=f"vps{t}")
            for ct in range(NCT):
                nc.tensor.matmul(
                    out=psv,
                    lhsT=xt_sb[ct][:, ts(t, 128)],
                    rhs=wv_sb[:, ct, :],
                    start=(ct == 0),
                    stop=False,
                )
            nc.tensor.matmul(
                out=psv, lhsT=ones_sb, rhs=bv_sb, start=False, stop=True
            )
            nc.vector.tensor_copy(
                out=v_sb[t][:, :, 0:D],
                in_=psv.rearrange("p (h d) -> p h d", h=HL),
            )
            nc.gpsimd.memset(v_sb[t][:, :, D : D + 1], 1.0)

        def y_block(qb, eng_toggle=[0]):
            """Output projection for query block qb (all pairs done)."""
            for qi in range(4 * qb, 4 * qb + 4):
                y_sb = ypool.tile([128, C], F32, tag="y", name=f"y{qi}")
                for half in range(2):
                    fps = psum.tile(
                        [128, FL], F32, tag="fill", bufs=2, name=f"fps{qi}_{half}"
                    )
                    for pair in range(NPAIR):
                        nc.tensor.matmul(
                            out=fps,
                            lhsT=otn_sb[pair][:, ts(qi, 128)],
                            rhs=wp_sb[:, pair, ts(half, FL)],
                            start=(pair == 0),
                            stop=(pair == NPAIR - 1),
                        )
                    nc.vector.tensor_copy(out=y_sb[:, ts(half, FL)], in_=fps)
                nc.sync.dma_start(out=y_d[ts(qi, 128), :], in_=y_sb)

        # ---- front: transposes + V + K/Q(pair 0) ----
        for n in range(4):
            for tt in range(4):
                t = 4 * n + tt
                tp = psum.tile(
                    [128, C], BF16, tag="fill", bufs=2, name=f"tp{t}"
                )
                for ct in range(NCT):
                    nc.tensor.transpose(
                        tp[:, ts(ct, 128)], xb_sb[t][:, ts(ct, 128)], ident
                    )
                for ct in range(NCT):
                    # ACT engine is idle during the front phase
                    nc.scalar.copy(
                        out=xt_sb[ct][:, ts(t, 128)], in_=tp[:, ts(ct, 128)]
                    )
                v_chain(t)
            kq_chain(0, n, "k")
            kq_chain(0, n, "q")

        # ---- attention per pair; fillers keep the PE stream dense ----
        def attention(pair):
            # filler work queue: list of thunks issued every few kt-steps
            fillers = []
            if pair < NPAIR - 1:
                for n in range(4):
                    fillers.append(lambda n=n: kq_chain(pair + 1, n, "k"))
                    fillers.append(lambda n=n: kq_chain(pair + 1, n, "q"))
            fi = [0]

            def pump_filler():
                if fi[0] < len(fillers):
                    fillers[fi[0]]()
                    fi[0] += 1

            hs = (2 * pair, 2 * pair + 1)
            step = 0
            for qb in range(NQB):
                q0 = 512 * qb
                nkt = 4 * qb + 4
                ot_h = {
                    h: psum.tile(
                        [128, 512], F32, tag="ot", bufs=2, name=f"ot{h}_{qb}"
                    )
                    for h in hs
                }
                prev = None  # (pb per head, off) of previous kt
                for kt in range(nkt):
                    c0 = 128 * kt
                    off = max(0, c0 - q0)
                    diag = c0 >= q0
                    cur = {}
                    for h in hs:
                        row0 = 64 * (h % 2)
                        sps = psum.tile(
                            [128, 512], F32, tag="sps", bufs=4,
                            name=f"s{h}_{qb}_{kt}",
                        )
                        nc.tensor.matmul(
                            out=sps[:, off:512],
                            lhsT=kt_sb[pair][row0 : row0 + 64, ts(kt, 128)],
                            rhs=qt_sb[pair][row0 : row0 + 64, q0 + off : q0 + 512],
                            start=True,
                            stop=not diag,
                            tile_position=(row0, 0),
                            skip_group_check=True,
                        )
                        if diag:
                            # causal mask of the diagonal 128x128 block,
                            # accumulated by the PE itself
                            nc.tensor.matmul(
                                out=sps[:, off : off + 128],
                                lhsT=ident,
                                rhs=trimask,
                                start=False,
                                stop=True,
                                skip_group_check=True,
                            )
                        pb = ppool.tile(
                            [128, 512], BF16, tag="p", name=f"p{h}_{qb}_{kt}"
                        )
                        nc.scalar.activation(
                            out=pb[:, off:512],
                            in_=sps[:, off:512],
                            func=mybir.ActivationFunctionType.Exp,
                            scale=float(D) ** -0.5,
                        )
                        cur[h] = (pb, off)
                    if prev is not None:
                        pkt = kt - 1
                        for h in hs:
                            pb, poff = prev[h]
                            nc.tensor.matmul(
                                out=ot_h[h][0 : D + 1, poff:512],
                                lhsT=v_sb[pkt][:, h, :],
                                rhs=pb[:, poff:512],
                                start=(pkt == 0),
                                stop=False,
                                skip_group_check=True,
                            )
                    prev = cur
                    step += 1
                    if step % 5 == 0:
                        pump_filler()
                # flush last kt's OT
                pkt = nkt - 1
                for h in hs:
                    pb, poff = prev[h]
                    nc.tensor.matmul(
                        out=ot_h[h][0 : D + 1, poff:512],
                        lhsT=v_sb[pkt][:, h, :],
                        rhs=pb[:, poff:512],
                        start=(pkt == 0),
                        stop=True,
                        skip_group_check=True,
                    )
                # normalize: r = 1/l (row D of ot psum), otn = O~ * r
                for h in hs:
                    row0 = 64 * (h % 2)
                    otps = ot_h[h]
                    lv = small.tile([1, 512], F32, tag="l", name=f"l{h}_{qb}")
                    nc.vector.tensor_copy(out=lv, in_=otps[D : D + 1, :])
                    rv = small.tile([1, 512], F32, tag="r", name=f"r{h}_{qb}")
                    nc.vector.reciprocal_approx_fast(out=rv, in_=lv)
                    rb = small.tile([64, 512], F32, tag="R", name=f"R{h}_{qb}")
                    nc.gpsimd.partition_broadcast(rb, rv)
                    nc.vector.tensor_mul(
                        otn_sb[pair][row0 : row0 + 64, ts(qb, 512)],
                        otps[0:D, :],
                        rb,
                    )
                if pair == NPAIR - 1 and qb > 0:
                    # previous query block now complete across all pairs
                    y_block(qb - 1)
            # drain leftover fillers
            while fi[0] < len(fillers):
                pump_filler()

        for pair in range(NPAIR):
            attention(pair)
        y_block(NQB - 1)

    nc.compile()
    return nc


_NC = None


def _get_nc():
    global _NC
    if _NC is None:
        _NC = build_nc()
    return _NC


def make_in_maps(x, W_kqv, b_kqv, W_proj):
    ki = np.arange(128)[:, None]
    qi = np.arange(128)[None, :]
    mask = np.where(ki <= qi, 0.0, MASK_NEG).astype(np.float32)
    in_maps = []
    for core in range(8):
        b = core // 2
        h0 = (core % 2) * HL * D  # feature offset of this core's head group
        in_maps.append(
            {
                "x": np.ascontiguousarray(x[b]),
                "wk": np.ascontiguousarray(W_kqv[:, h0 : h0 + FL]),
                "wq": np.ascontiguousarray(W_kqv[:, C + h0 : C + h0 + FL]),
                "wv": np.ascontiguousarray(W_kqv[:, 2 * C + h0 : 2 * C + h0 + FL]),
                "wp": np.ascontiguousarray(W_proj[h0 : h0 + FL, :]),
                "bk": np.ascontiguousarray(b_kqv[h0 : h0 + FL]),
                "bq": np.ascontiguousarray(b_kqv[C + h0 : C + h0 + FL]),
                "bv": np.ascontiguousarray(b_kqv[2 * C + h0 : 2 * C + h0 + FL]),
                "mask": mask,
            }
        )
    return in_maps


def _combine(results, b_proj):
    y = np.empty((B, T, C), dtype=np.float32)
    for b in range(B):
        y[b] = results[2 * b]["y"] + results[2 * b + 1]["y"] + b_proj[None, :]
    return y


def kernel(x, W_kqv, b_kqv, W_proj, b_proj, **run_kwargs):
    x = np.asarray(x, dtype=np.float32)
    W_kqv = np.asarray(W_kqv, dtype=np.float32)
    b_kqv = np.asarray(b_kqv, dtype=np.float32)
    W_proj = np.asarray(W_proj, dtype=np.float32)
    b_proj = np.asarray(b_proj, dtype=np.float32)

    nc = _get_nc()
    in_maps = make_in_maps(x, W_kqv, b_kqv, W_proj)
    res = run_bass_kernel_spmd(nc, in_maps, core_ids=list(range(8)), **run_kwargs)
    out = _combine(res.results, b_proj)
    kernel.last_result = res
    return out


# revision 7
# speedup vs baseline: 1.0489x; 1.0489x over previous
"""Causal self-attention (B=4, T=2048, C=768, H=12) on 8 TRN2 NeuronCores.

Sharding: DP=4 over batch x TP=2 over heads (6 heads per core).

v3 design, built from HAM-throttle analysis of the v1/v2 traces:
  - The PE defaults to 1.2 GHz and only reaches 2.4 GHz under *sustained*
    busy-ness (PE_HAM 4096-cycle activity windows).  The kernel keeps the
    PE stream dense: S / OT matmuls for the two heads of a pair are
    software-pipelined (OT lags one k-tile pair behind S), and idle slack
    is filled with independent work (late V tiles and the next pair's kqv
    chains during earlier pairs, output projection of completed query
    blocks during the last pair).
  - x^T transposes on the PE (matmul-transpose vs identity) instead of the
    serialized sync-DMA queue; PSUM drains split across scalar + vector.
  - The causal diagonal mask is accumulated into scores by a PE matmul
    (identity x trimask).
  - exp() runs on the scalar engine (the only engine with ACT tables);
    scores for two consecutive k-tiles are packed side by side in one
    [128, 1024] PSUM tile so each ACT instruction is maximally wide
    (~13.4M causal-area elements; per-instruction overhead ~25%).

Per-core pipeline:
    front:  x tiles (gpsimd cast-DMA) -> PE transpose -> X^T,
            V tiles 0-7, K/Q pair 0
    pair p: for qb (512 queries): for kt-pair: S^T(h0) x2, S^T(h1) x2
            [+ diag mask matmuls], exp -> P (bf16), OT(h, prev pair)
            accumulating [O~^T; l] over kt; normalize rows by 1/l.
    Y: y[qb] = sum_p otn[p]' Wp[p]  (PSUM chain over pairs) -> DMA out.
Host sums the two TP partials per batch and adds b_proj.
"""

import sys

sys.path.insert(0, "/opt/trn_rl_repo")

from contextlib import ExitStack

import numpy as np

import concourse.bass as bass
import concourse.tile as tile
from concourse import bacc
from concourse import mybir
from concourse.bass import ts
from concourse.bass_utils import run_bass_kernel_spmd
from concourse.masks import make_identity

F32 = mybir.dt.float32
BF16 = mybir.dt.bfloat16

B, T, C = 4, 2048, 768
H, D = 12, 64
HL = 6          # heads per core
FL = HL * D     # 384 local feature dim
NCT = C // 128  # 6 contraction tiles
NT = T // 128   # 16 token tiles
NQB = T // 512  # 4 query blocks
NPAIR = HL // 2  # 3 head pairs

MASK_NEG = -30000.0


def build_nc():
    nc = bacc.Bacc()
    x_d = nc.declare_dram_parameter("x", [T, C], F32, isOutput=False)
    wk_d = nc.declare_dram_parameter("wk", [C, FL], F32, isOutput=False)
    wq_d = nc.declare_dram_parameter("wq", [C, FL], F32, isOutput=False)
    wv_d = nc.declare_dram_parameter("wv", [C, FL], F32, isOutput=False)
    wp_d = nc.declare_dram_parameter("wp", [FL, C], F32, isOutput=False)
    bk_d = nc.declare_dram_parameter("bk", [FL], F32, isOutput=False)
    bq_d = nc.declare_dram_parameter("bq", [FL], F32, isOutput=False)
    bv_d = nc.declare_dram_parameter("bv", [FL], F32, isOutput=False)
    mask_d = nc.declare_dram_parameter("mask", [128, 128], F32, isOutput=False)
    y_d = nc.declare_dram_parameter("y", [T, C], F32, isOutput=True)

    with tile.TileContext(nc) as tc, ExitStack() as ctx:
        const = ctx.enter_context(tc.tile_pool(name="const", bufs=1))
        wpool = ctx.enter_context(tc.tile_pool(name="wpool", bufs=1))
        big = ctx.enter_context(tc.tile_pool(name="big", bufs=1))
        xin = ctx.enter_context(tc.tile_pool(name="xin", bufs=6))
        ppool = ctx.enter_context(tc.tile_pool(name="ppool", bufs=6))
        small = ctx.enter_context(tc.tile_pool(name="small", bufs=2))
        ypool = ctx.enter_context(tc.tile_pool(name="ypool", bufs=4))
        psum = ctx.enter_context(tc.tile_pool(name="psum", bufs=2, space="PSUM"))

        # ---- constants / weights / inputs ----
        ident = const.tile([128, 128], BF16)
        trimask = const.tile([128, 128], BF16)
        ones_sb = const.tile([1, 128], BF16)
        bk_sb = const.tile([128, NPAIR], F32)
        bq_sb = const.tile([128, NPAIR], F32)
        bv_sb = const.tile([1, FL], BF16)
        wk_sb = wpool.tile([128, NCT, FL], BF16)
        wq_sb = wpool.tile([128, NCT, FL], BF16)
        wv_sb = wpool.tile([128, NCT, FL], BF16)
        wp_sb = wpool.tile([128, NPAIR, C], BF16)
        xb_sb = [
            xin.tile([128, C], BF16, tag="xb", bufs=NT, name=f"xb{t}")
            for t in range(NT)
        ]

        # gpsimd (SWDGE) queue order = availability order; casts f32->bf16
        # in flight.  x0/x1 + wv/wk first so the PE front can start early.
        nc.gpsimd.dma_start(out=xb_sb[0], in_=x_d[ts(0, 128), :])
        nc.gpsimd.dma_start(out=xb_sb[1], in_=x_d[ts(1, 128), :])
        make_identity(nc, ident)
        nc.vector.memset(ones_sb, 1.0)
        nc.gpsimd.dma_start(
            out=wv_sb, in_=wv_d.rearrange("(a p) f -> p a f", p=128)
        )
        nc.gpsimd.dma_start(
            out=wk_sb, in_=wk_d.rearrange("(a p) f -> p a f", p=128)
        )
        nc.gpsimd.dma_start(out=bv_sb, in_=bv_d.rearrange("(o f) -> o f", o=1))
        nc.gpsimd.dma_start(out=xb_sb[2], in_=x_d[ts(2, 128), :])
        nc.gpsimd.dma_start(out=xb_sb[3], in_=x_d[ts(3, 128), :])
        nc.gpsimd.dma_start(
            out=wq_sb, in_=wq_d.rearrange("(a p) f -> p a f", p=128)
        )
        nc.gpsimd.dma_start(out=bk_sb, in_=bk_d.rearrange("(i p) -> p i", p=128))
        nc.gpsimd.dma_start(out=bq_sb, in_=bq_d.rearrange("(i p) -> p i", p=128))
        for t in range(4, 8):
            nc.gpsimd.dma_start(out=xb_sb[t], in_=x_d[ts(t, 128), :])
        nc.gpsimd.dma_start(
            out=wp_sb, in_=wp_d.rearrange("(a p) f -> p a f", p=128)
        )
        nc.gpsimd.dma_start(out=trimask, in_=mask_d[:, :])
        for t in range(8, 16):
            nc.gpsimd.dma_start(out=xb_sb[t], in_=x_d[ts(t, 128), :])

        # ---- persistent activations ----
        xt_sb = [
            big.tile([128, T], BF16, tag="xt", bufs=NCT, name=f"xt{ct}")
            for ct in range(NCT)
        ]
        kt_sb = [
            big.tile([128, T], BF16, tag="ktq", bufs=2 * NPAIR, name=f"ktp{i}")
            for i in range(NPAIR)
        ]
        qt_sb = [
            big.tile([128, T], BF16, tag="ktq", bufs=2 * NPAIR, name=f"qtp{i}")
            for i in range(NPAIR)
        ]
        v_sb = [
            big.tile([128, HL, D + 1], BF16, tag="v", bufs=NT, name=f"v{t}")
            for t in range(NT)
        ]
        otn_sb = [
            big.tile([128, T], BF16, tag="otn", bufs=NPAIR, name=f"otn{i}")
            for i in range(NPAIR)
        ]

        # ---- helpers ----
        def kq_chain(pair, n, which, ptag):
            """One K- or Q-pair projection chain for token block n (512)."""
            w_src, dest, bias = (
                (wk_sb, kt_sb, bk_sb) if which == "k" else (wq_sb, qt_sb, bq_sb)
            )
            ps = psum.tile(
                [128, 512], F32, tag=ptag, bufs=2, name=f"kq{which}{pair}_{n}"
            )
            for ct in range(NCT):
                nc.tensor.matmul(
                    out=ps,
                    lhsT=w_src[:, ct, ts(pair, 128)],
                    rhs=xt_sb[ct][:, ts(n, 512)],
                    start=(ct == 0),
                    stop=(ct == NCT - 1),
                )
            nc.vector.tensor_scalar_add(
                out=dest[pair][:, ts(n, 512)], in0=ps, scalar1=bias[:, pair : pair + 1]
            )

        def v_chain(t, ptag):
            """V (natural layout) for token tile t, bias via ones-row matmul."""
            psv = psum.tile([128, FL], F32, tag=ptag, bufs=2, name=f"vps{t}")
            for ct in range(NCT):
                nc.tensor.matmul(
                    out=psv,
                    lhsT=xt_sb[ct][:, ts(t, 128)],
                    rhs=wv_sb[:, ct, :],
                    start=(ct == 0),
                    stop=False,
                )
            nc.tensor.matmul(
                out=psv, lhsT=ones_sb, rhs=bv_sb, start=False, stop=True
            )
            nc.vector.tensor_copy(
                out=v_sb[t][:, :, 0:D],
                in_=psv.rearrange("p (h d) -> p h d", h=HL),
            )
            nc.gpsimd.memset(v_sb[t][:, :, D : D + 1], 1.0)

        def y_half(qi, half, y_sb):
            fps = psum.tile(
                [128, FL], F32, tag="fill", bufs=2, name=f"fps{qi}_{half}"
            )
            for pair in range(NPAIR):
                nc.tensor.matmul(
                    out=fps,
                    lhsT=otn_sb[pair][:, ts(qi, 128)],
                    rhs=wp_sb[:, pair, ts(half, FL)],
                    start=(pair == 0),
                    stop=(pair == NPAIR - 1),
                )
            nc.vector.tensor_copy(out=y_sb[:, ts(half, FL)], in_=fps)
            if half == 1:
                nc.sync.dma_start(out=y_d[ts(qi, 128), :], in_=y_sb)

        def y_thunks(qb):
            """Output projection thunks for query block qb (all pairs done)."""
            out = []
            for qi in range(4 * qb, 4 * qb + 4):
                y_sb = ypool.tile([128, C], F32, tag="y", name=f"y{qi}")
                for half in range(2):
                    out.append(lambda qi=qi, half=half, y_sb=y_sb: y_half(qi, half, y_sb))
            return out

        # ---- front: transposes + V(0..7) + K/Q(pair 0) ----
        for n in range(4):
            for tt in range(4):
                t = 4 * n + tt
                tp = psum.tile(
                    [128, C], BF16, tag="fill", bufs=2, name=f"tp{t}"
                )
                for ct in range(NCT):
                    nc.tensor.transpose(
                        tp[:, ts(ct, 128)], xb_sb[t][:, ts(ct, 128)], ident
                    )
                for ct in range(NCT):
                    # ACT engine is idle during the front phase
                    nc.scalar.copy(
                        out=xt_sb[ct][:, ts(t, 128)], in_=tp[:, ts(ct, 128)]
                    )
                if t < 8:
                    v_chain(t, "ot")
            kq_chain(0, n, "k", "ot")
            kq_chain(0, n, "q", "ot")

        # ---- attention per pair; fillers keep the PE stream dense ----
        def attention(pair):
            if pair == 0:
                fillers = [lambda t=t: v_chain(t, "fill") for t in range(8, 16)]
                fillers += [
                    lambda n=n, w=w: kq_chain(1, n, w, "fill")
                    for n in range(4)
                    for w in ("k", "q")
                ]
            elif pair == 1:
                fillers = [
                    lambda n=n, w=w: kq_chain(2, n, w, "fill")
                    for n in range(4)
                    for w in ("k", "q")
                ]
            else:
                fillers = []  # y thunks appended as query blocks complete
            fi = [0]

            def pump_filler():
                if fi[0] < len(fillers):
                    fillers[fi[0]]()
                    fi[0] += 1

            hs = (2 * pair, 2 * pair + 1)
            for qb in range(NQB):
                q0 = 512 * qb
                npk = 2 * qb + 2  # kt pairs in this query block
                ot_h = {
                    h: psum.tile(
                        [128, 512], F32, tag="ot", bufs=2, name=f"ot{h}_{qb}"
                    )
                    for h in hs
                }
                prev = None  # per head: (pb, off_even, off_odd, kt_even)
                for kp in range(npk):
                    kt0, kt1 = 2 * kp, 2 * kp + 1
                    # offs: valid-query start within the block, per k-tile
                    o0 = max(0, 128 * kt0 - q0)
                    o1 = max(0, 128 * kt1 - q0)
                    diag = 128 * kt1 >= q0
                    cur = {}
                    for h in hs:
                        row0 = 64 * (h % 2)
                        sps = psum.tile(
                            [128, 1024], F32, tag="sps", bufs=2,
                            name=f"s{h}_{qb}_{kp}",
                        )
                        for sub, kt in ((0, kt0), (1, kt1)):
                            # both halves start at o0 so the exp span below
                            # reads only initialized PSUM
                            nc.tensor.matmul(
                                out=sps[:, 512 * sub + o0 : 512 * sub + 512],
                                lhsT=kt_sb[pair][row0 : row0 + 64, ts(kt, 128)],
                                rhs=qt_sb[pair][
                                    row0 : row0 + 64, q0 + o0 : q0 + 512
                                ],
                                start=True,
                                stop=not (diag and 128 * kt >= q0),
                                tile_position=(row0, 0),
                                skip_group_check=True,
                            )
                            if diag and 128 * kt >= q0:
                                off = 128 * kt - q0
                                nc.tensor.matmul(
                                    out=sps[:, 512 * sub + off : 512 * sub + off + 128],
                                    lhsT=ident,
                                    rhs=trimask,
                                    start=False,
                                    stop=True,
                                    skip_group_check=True,
                                )
                        pb = ppool.tile(
                            [128, 1024], BF16, tag="p", name=f"p{h}_{qb}_{kp}"
                        )
                        nc.scalar.activation(
                            out=pb[:, o0:1024],
                            in_=sps[:, o0:1024],
                            func=mybir.ActivationFunctionType.Exp,
                            scale=float(D) ** -0.5,
                        )
                        cur[h] = (pb, o0, o1, kt0)
                    if prev is not None:
                        for h in hs:
                            pb, po0, po1, pkt = prev[h]
                            for sub, kt, off in ((0, pkt, po0), (1, pkt + 1, po1)):
                                nc.tensor.matmul(
                                    out=ot_h[h][0 : D + 1, off:512],
                                    lhsT=v_sb[kt][:, h, :],
                                    rhs=pb[:, 512 * sub + off : 512 * sub + 512],
                                    start=(kt == 0),
                                    stop=False,
                                    skip_group_check=True,
                                )
                    prev = cur
                    pump_filler()
                # flush last kt pair's OT
                for h in hs:
                    pb, po0, po1, pkt = prev[h]
                    for sub, kt, off in ((0, pkt, po0), (1, pkt + 1, po1)):
                        nc.tensor.matmul(
                            out=ot_h[h][0 : D + 1, off:512],
                            lhsT=v_sb[kt][:, h, :],
                            rhs=pb[:, 512 * sub + off : 512 * sub + 512],
                            start=(kt == 0),
                            stop=(sub == 1),
                            skip_group_check=True,
                        )
                # normalize: r = 1/l (row D of ot psum), otn = O~ * r
                for h in hs:
                    row0 = 64 * (h % 2)
                    otps = ot_h[h]
                    lv = small.tile([1, 512], F32, tag="l", name=f"l{h}_{qb}")
                    nc.vector.tensor_copy(out=lv, in_=otps[D : D + 1, :])
                    rv = small.tile([1, 512], F32, tag="r", name=f"r{h}_{qb}")
                    nc.vector.reciprocal_approx_fast(out=rv, in_=lv)
                    rb = small.tile([64, 512], F32, tag="R", name=f"R{h}_{qb}")
                    nc.gpsimd.partition_broadcast(rb, rv)
                    nc.vector.tensor_mul(
                        otn_sb[pair][row0 : row0 + 64, ts(qb, 512)],
                        otps[0:D, :],
                        rb,
                    )
                if pair == NPAIR - 1 and qb < NQB - 1:
                    # previous query block now complete across all pairs
                    fillers.extend(y_thunks(qb))
            # drain leftover fillers
            while fi[0] < len(fillers):
                pump_filler()

        for pair in range(NPAIR):
            attention(pair)
        for th in y_thunks(NQB - 1):
            th()

    nc.compile()
    return nc


_NC = None


def _get_nc():
    global _NC
    if _NC is None:
        _NC = build_nc()
    return _NC


def make_in_maps(x, W_kqv, b_kqv, W_proj):
    ki = np.arange(128)[:, None]
    qi = np.arange(128)[None, :]
    mask = np.where(ki <= qi, 0.0, MASK_NEG).astype(np.float32)
    in_maps = []
    for core in range(8):
        b = core // 2
        h0 = (core % 2) * HL * D  # feature offset of this core's head group
        in_maps.append(
            {
                "x": np.ascontiguousarray(x[b]),
                "wk": np.ascontiguousarray(W_kqv[:, h0 : h0 + FL]),
                "wq": np.ascontiguousarray(W_kqv[:, C + h0 : C + h0 + FL]),
                "wv": np.ascontiguousarray(W_kqv[:, 2 * C + h0 : 2 * C + h0 + FL]),
                "wp": np.ascontiguousarray(W_proj[h0 : h0 + FL, :]),
                "bk": np.ascontiguousarray(b_kqv[h0 : h0 + FL]),
                "bq": np.ascontiguousarray(b_kqv[C + h0 : C + h0 + FL]),
                "bv": np.ascontiguousarray(b_kqv[2 * C + h0 : 2 * C + h0 + FL]),
                "mask": mask,
            }
        )
    return in_maps


def _combine(results, b_proj):
    y = np.empty((B, T, C), dtype=np.float32)
    for b in range(B):
        y[b] = results[2 * b]["y"] + results[2 * b + 1]["y"] + b_proj[None, :]
    return y


def kernel(x, W_kqv, b_kqv, W_proj, b_proj, **run_kwargs):
    x = np.asarray(x, dtype=np.float32)
    W_kqv = np.asarray(W_kqv, dtype=np.float32)
    b_kqv = np.asarray(b_kqv, dtype=np.float32)
    W_proj = np.asarray(W_proj, dtype=np.float32)
    b_proj = np.asarray(b_proj, dtype=np.float32)

    nc = _get_nc()
    in_maps = make_in_maps(x, W_kqv, b_kqv, W_proj)
    res = run_bass_kernel_spmd(nc, in_maps, core_ids=list(range(8)), **run_kwargs)
    out = _combine(res.results, b_proj)
    kernel.last_result = res
    return out
